# revision 6
# baseline (speedup 1.0000x reference)
"""Trainium2 Bass kernel v2 for Linformer self-attention (ragged projection).

Per batch sample b (data-parallel over 8 cores), with K=256 << S=4096 the
k/v projections are reordered to project h down to K rows FIRST:

    L        = sum(mask > -1);  mm[s] = (mask[s] > -1) / sqrt(L)
    pkvm     = [pk | pv] * mm[:, None]                  # [S, 2K]
    hpT      = h.T @ pkvm                               # [D, 2K]   phase 1
    cs       = ones.T @ (pkv * mm)                      # [1, 2K]   (bias csum)
    kT       = Wk.T.T @ hpT[:, :K] + bk x csk           # [D, K]    phase 2k
    v        = hpT[:, K:].T @ Wv.T + csv x bv           # [K, D]    phase 2v
    qT       = (Wq.T/sqrt(DH)).T @ h.T + bq/sqrt(DH)    # [D, S]    per group
    per head i:  scT = kT_i.T @ qT_i    [K, 512]        # heads row-packed x2
                 probT = exp(scT)  (unnormalized, bf16)
                 ctx[s, 64i+j] = probT.T @ [v_i | 1]; normalize by col 64

This cuts matmul flops ~1.9x vs computing full [S,D] hk/hv. All matmul
inputs are bf16 (f32 PSUM accumulation); DMA volume is halved vs f32.
ctx is computed in [s, d] layout (probT chunks as the stationary operand)
so the softmax denominator lands per-partition -> cheap tensor_scalar
normalization and contiguous output DMA.
"""

import numpy as np

import concourse.mybir as mybir
import concourse.tile as tile
from concourse import bacc
from concourse import bass_utils

P = 128
f32 = mybir.dt.float32
bf16 = mybir.dt.bfloat16
AF = mybir.ActivationFunctionType
ALU = mybir.AluOpType

# Problem dims (nn_LinformerSelfAttention): B=8, S=4096, D=1024, H=16, K=256
B = 8
S_FULL = 4096
D_FULL = 1024
KL_FULL = 256
DH = 64


def _blob_offsets(S, D, KL):
    """Element offsets of each packed input inside the two input blobs."""
    KV = 2 * KL
    offs = {}
    o = 0
    for name, n in [("hS", S * D), ("hT", D * S), ("pkv", S * KV),
                    ("wqT", D * D), ("wkT", D * D), ("wvT", D * D),
                    ("bkr", D), ("bvr", D)]:
        offs[name] = o
        o += n
    offs["nb16"] = o
    o = 0
    for name, n in [("mask", S), ("bqs", D)]:
        offs[name] = o
        o += n
    offs["nb32"] = o
    return offs


def build_program(S=S_FULL, D=D_FULL, KL=KL_FULL, ablate=None):
    """ablate: None=full, 'p12'=stop after phase2, 'q'=+q (no attention),
    'nosc'=+scores/exp (no ctx/norm/out)."""
    SC = S // P           # s-chunks of 128 (32)
    SG = S // 512         # s-groups of 512 (8)
    DC = D // P           # d-chunks of 128 (8)
    KC = KL // P          # linformer-k chunks of 128 (2)
    KV = 2 * KL           # fused [pk|pv] width (512)
    H = D // DH           # heads (16)
    HP = H // 2           # head pairs per d-chunk
    assert S % 512 == 0 and D % P == 0 and KL % P == 0 and KV <= 512

    nc = bacc.Bacc("TRN2", target_bir_lowering=False, debug=False)

    # All inputs are packed into two blobs (one bf16, one f32) to minimize
    # the per-execution dispatch cost, which scales with buffer count.
    offs = _blob_offsets(S, D, KL)
    blob16 = nc.dram_tensor("blob16", [offs["nb16"]], bf16, kind="ExternalInput")
    blob32 = nc.dram_tensor("blob32", [offs["nb32"]], f32, kind="ExternalInput")
    out = nc.dram_tensor("out", [S, D], f32, kind="ExternalOutput")

    def b16(name, n):
        o = offs[name]
        return blob16.ap()[o:o + n]

    def b32(name, n):
        o = offs[name]
        return blob32.ap()[o:o + n]

    with tile.TileContext(nc) as tc:
        with (
            tc.tile_pool(name="persist", bufs=1) as persist,
            tc.tile_pool(name="wpool", bufs=1) as wpool,
            tc.tile_pool(name="hpool", bufs=2) as hpool,
            tc.tile_pool(name="spool", bufs=4) as spool,
            tc.tile_pool(name="qpool", bufs=2) as qpool,
            tc.tile_pool(name="cpool", bufs=2) as cpool,
        ):
            # ---------- setup: mask stats ----------
            with tc.tile_pool(name="psetup", bufs=1, space="PSUM") as psetup:
                mt = spool.tile([P, SC], f32, tag="mt")
                nc.sync.dma_start(out=mt[:], in_=b32("mask", S).rearrange("(c p) -> p c", p=P))
                m01 = spool.tile([P, SC], f32, tag="m01")
                nc.vector.tensor_scalar(m01[:], mt[:], -1.0, None, ALU.is_gt)
                lp = spool.tile([P, 1], f32, tag="lp")
                nc.vector.tensor_reduce(lp[:], m01[:], mybir.AxisListType.X, ALU.add)
                ones_col = persist.tile([P, 1], bf16, tag="onc")
                nc.vector.memset(ones_col[:], 1.0)
                ones_colf = spool.tile([P, 1], f32, tag="oncf")
                nc.vector.memset(ones_colf[:], 1.0)
                ones_row = spool.tile([1, P], f32, tag="onr")
                nc.vector.memset(ones_row[:], 1.0)
                lps = psetup.tile([1, 1], f32)
                nc.tensor.matmul(lps[:], lp[:], ones_colf[:], start=True, stop=True)
                lrec = spool.tile([1, 1], f32, tag="lrec")
                nc.vector.reciprocal(lrec[:], lps[:])
                inv = spool.tile([1, 1], f32, tag="inv")
                nc.scalar.activation(inv[:], lrec[:], AF.Sqrt)
                invps = psetup.tile([P, 1], f32)
                nc.tensor.matmul(invps[:], ones_row[:], inv[:], start=True, stop=True)
                invcol = spool.tile([P, 1], f32, tag="invcol")
                nc.vector.tensor_copy(invcol[:], invps[:])
                # mm = (mask > -1) / sqrt(L), per-s column layout [P, SC]
                mm_sb = persist.tile([P, SC], f32, tag="mmsb")
                nc.vector.tensor_scalar(mm_sb[:], m01[:], invcol[:], None, ALU.mult)

            # ---------- setup: biases ----------
            bq_sb = persist.tile([P, DC], f32, tag="bqsb")
            nc.sync.dma_start(out=bq_sb[:], in_=b32("bqs", D).rearrange("(m p) -> p m", p=P))
            bk_row = persist.tile([1, D], bf16, tag="bkrow")
            nc.sync.dma_start(out=bk_row[:], in_=b16("bkr", D)[None, :])
            bv_row = persist.tile([1, D], bf16, tag="bvrow")
            nc.sync.dma_start(out=bv_row[:], in_=b16("bvr", D)[None, :])

            # ---------- phase 1: hpT[d, :] = sum_s h[s, d] * pkvm[s, :] -------
            # pkv chunks stream interleaved with hS chunks so the s-loop is
            # PE-bound from the start; pkv stays resident for the cs pass.
            pkv_sb = wpool.tile([P, SC, KV], bf16, tag="pkvsb", name="pkvsb")
            hp_sb = persist.tile([P, DC, KV], bf16, tag="hpsb")
            with tc.tile_pool(name="p1", bufs=1, space="PSUM") as p1:
                hp_ps = [p1.tile([P, KV], f32, tag=f"hp{d}", name=f"hpps{d}")
                         for d in range(DC)]
                csacc = persist.tile([P, KV], f32, tag="csacc")
                for G in range(SC // 4):
                    nc.sync.dma_start(
                        out=pkv_sb[:, 4 * G:4 * (G + 1), :],
                        in_=blob16.ap()[offs["pkv"] + 512 * G * KV:
                                        offs["pkv"] + 512 * (G + 1) * KV]
                        .rearrange("(c p k) -> p c k", p=P, k=KV))
                    h_g = hpool.tile([P, 4, D], bf16, tag="hc")
                    nc.sync.dma_start(
                        out=h_g[:],
                        in_=blob16.ap()[offs["hS"] + 512 * G * D:
                                        offs["hS"] + 512 * (G + 1) * D]
                        .rearrange("(c p d) -> p c d", p=P, d=D))
                    for c in range(4):
                        s = 4 * G + c
                        pkvm = spool.tile([P, KV], bf16, tag="pkvm")
                        nc.vector.tensor_scalar(pkvm[:], pkv_sb[:, s, :],
                                                mm_sb[:, s:s + 1], None, ALU.mult)
                        for d in range(DC):
                            nc.tensor.matmul(hp_ps[d][:], h_g[:, c, P * d:P * (d + 1)],
                                             pkvm[:], start=(s == 0),
                                             stop=(s == SC - 1))
                        # per-partition partial csums for the bias rank-1 term
                        if s == 0:
                            nc.vector.tensor_copy(csacc[:], pkvm[:])
                        else:
                            nc.vector.tensor_tensor(csacc[:], csacc[:], pkvm[:],
                                                    ALU.add)
                for d in range(DC):
                    nc.vector.tensor_copy(hp_sb[:, d, :], hp_ps[d][:])

            # phase-2/q weights: DMAs queue behind the hS stream, land during
            # the phase-1 compute.
            wk_sb = wpool.tile([P, DC, D], bf16, tag="wk", name="wksb")
            wv_sb = wpool.tile([P, DC, D], bf16, tag="wv", name="wvsb")
            nc.sync.dma_start(out=wk_sb[:],
                              in_=b16("wkT", D * D)
                              .rearrange("(c p d) -> p c d", p=P, d=D))
            nc.sync.dma_start(out=wv_sb[:],
                              in_=b16("wvT", D * D)
                              .rearrange("(c p d) -> p c d", p=P, d=D))

            # ---------- cs = [csk | csv] = sum_s mm[s] * pkv[s, :] ------------
            # partition-reduce the DVE-accumulated csacc with one matmul
            cs_sb = persist.tile([1, KV], bf16, tag="cssb")
            csacc_bf = spool.tile([P, KV], bf16, tag="csaccbf")
            nc.vector.tensor_copy(csacc_bf[:], csacc[:])
            with tc.tile_pool(name="pcs", bufs=1, space="PSUM") as pcs:
                cs_ps = pcs.tile([1, KV], f32, name="csps")
                nc.tensor.matmul(cs_ps[:], ones_col[:], csacc_bf[:],
                                 start=True, stop=True)
                nc.vector.tensor_copy(cs_sb[:], cs_ps[:])

            # ---------- phase 2k: kT[d, k] = sum_d' wkT[d', d] hpT[d', k] ----
            #            (+ bk[d] * csk[k])
            kt_sb = persist.tile([P, DC, KL], bf16, tag="ktsb")
            # ---------- phase 2v: v[k, d] = sum_d' hpT[d', K+k] wvT[d', d] ---
            #            (+ csv[k] * bv[d]) ; stored as vaug [k, dh+1] per head
            vaug = persist.tile([P, H * KC, DH + 1], bf16, tag="vaug")
            nc.vector.memset(vaug[:, :, DH:DH + 1], 1.0)
            D5 = (D + 511) // 512  # 512-wide column groups of D
            with tc.tile_pool(name="p2", bufs=1, space="PSUM") as p2:
                kt_ps = [p2.tile([P, 2 * KL], f32, tag=f"kt{j}", name=f"ktps{j}")
                         for j in range(DC // 2)]
                v_ps = [p2.tile([P, 512], f32, tag=f"v{j}", name=f"vps{j}")
                        for j in range(KC * D5)]
                for d in range(DC):
                    o = kt_ps[d // 2][:, (d % 2) * KL:(d % 2) * KL + KL]
                    for dp in range(DC):
                        nc.tensor.matmul(o, wk_sb[:, dp, P * d:P * (d + 1)],
                                         hp_sb[:, dp, 0:KL],
                                         start=(dp == 0), stop=False)
                    nc.tensor.matmul(o, bk_row[:, P * d:P * (d + 1)],
                                     cs_sb[:, 0:KL], start=False, stop=True)
                for kc in range(KC):
                    for j in range(D5):
                        o = v_ps[kc * D5 + j][:]
                        for dp in range(DC):
                            nc.tensor.matmul(
                                o, hp_sb[:, dp, KL + P * kc:KL + P * (kc + 1)],
                                wv_sb[:, dp, 512 * j:512 * (j + 1)],
                                start=(dp == 0), stop=False)
                        nc.tensor.matmul(
                            o, cs_sb[:, KL + P * kc:KL + P * (kc + 1)],
                            bv_row[:, 512 * j:512 * (j + 1)],
                            start=False, stop=True)
                for d in range(DC):
                    nc.vector.tensor_copy(
                        kt_sb[:, d, :], kt_ps[d // 2][:, (d % 2) * KL:(d % 2) * KL + KL])
                for i in range(H):
                    j, off = divmod(DH * i, 512)
                    for kc in range(KC):
                        nc.vector.tensor_copy(vaug[:, i * KC + kc, 0:DH],
                                              v_ps[kc * D5 + j][:, off:off + DH])

            # ---------- q + attention, per 512-group ----------
            if ablate == "p12":
                # drain kt so the phase-2 pipeline has an observable effect
                nc.sync.dma_start(out=out.ap()[0:P, 0:KL // 2],
                                  in_=kt_sb[:, 0, :].bitcast(f32))
            wq_sb = wpool.tile([P, DC, D], bf16, tag="wq", name="wqsb")
            if ablate != "p12":
                nc.sync.dma_start(out=wq_sb[:],
                                  in_=b16("wqT", D * D)
                                  .rearrange("(c p d) -> p c d", p=P, d=D))
            with (
                tc.tile_pool(name="pq", bufs=2, space="PSUM") as pq,
                tc.tile_pool(name="psc", bufs=4, space="PSUM") as psc,
                tc.tile_pool(name="pctx", bufs=2, space="PSUM") as pctx,
            ):
                for g in range(SG if ablate != "p12" else 0):
                    ht_g = hpool.tile([P, DC, 512], bf16, tag="ht")
                    nc.sync.dma_start(
                        out=ht_g[:],
                        in_=b16("hT", D * S)
                        .rearrange("(c p s) -> p c s", p=P, s=S)
                        [:, :, 512 * g:512 * (g + 1)])
                    qt_g = qpool.tile([P, DC, 512], bf16, tag="qt")
                    for mq in range(DC):
                        q_ps = pq.tile([P, 512], f32, tag="qps")
                        for d in range(DC):
                            nc.tensor.matmul(q_ps[:],
                                             wq_sb[:, d, P * mq:P * (mq + 1)],
                                             ht_g[:, d, :],
                                             start=(d == 0), stop=(d == DC - 1))
                        # q + bq/sqrt(DH); bias varies along partitions.
                        # On DVE: the ACT engine is near-saturated with exp.
                        nc.vector.tensor_scalar(qt_g[:, mq, :], q_ps[:],
                                                bq_sb[:, mq:mq + 1], None, ALU.add)
                    ctx_g = cpool.tile([P, 4, D], f32, tag="ctxg")
                    # software-pipelined head pairs: scores(j) ahead of ctx(j-1)
                    sc_tiles = {}
                    prob_tiles = {}

                    def emit_scores_pair(t):
                        # heads (2t, 2t+1) live at partition offsets 0/64 of
                        # d-chunk t; interleave their MMs so the row-tiled
                        # halves of the PE array run them concurrently.
                        # One PSUM tile per (head, kc): buffers recycle at
                        # exp granularity, not head granularity.
                        mq = t
                        for kc in range(KC):
                            for hh in range(2):
                                po = DH * hh
                                sc = psc.tile([P, 512], f32, tag="sc",
                                              name=f"scps{hh}{kc}")
                                nc.tensor.matmul(
                                    sc[:],
                                    kt_sb[po:po + DH, mq, P * kc:P * (kc + 1)],
                                    qt_g[po:po + DH, mq, :], start=True, stop=True)
                                sc_tiles[(2 * t + hh, kc)] = sc

                    def emit_exp(j):
                        probT = spool.tile([P, KC, 512], bf16, tag="probT")
                        for kc in range(KC):
                            sc = sc_tiles.pop((j, kc))
                            nc.scalar.activation(probT[:, kc, :], sc[:],
                                                 AF.Exp)
                        prob_tiles[j] = probT

                    def emit_ctx(j):
                        i = j  # head index
                        probT = prob_tiles.pop(j)
                        ctx_ps = pctx.tile([P, 4, DH + 1], f32, tag="cx")
                        for c in range(4):
                            for kc in range(KC):
                                nc.tensor.matmul(
                                    ctx_ps[:, c, :],
                                    probT[:, kc, P * c:P * (c + 1)],
                                    vaug[:, i * KC + kc, :],
                                    start=(kc == 0), stop=(kc == KC - 1))
                        rec4 = spool.tile([P, 4, 1], f32, tag="rec4")
                        nc.vector.reciprocal(rec4[:], ctx_ps[:, :, DH:DH + 1])
                        nc.vector.tensor_tensor(
                            ctx_g[:, :, DH * i:DH * (i + 1)],
                            ctx_ps[:, :, 0:DH],
                            rec4[:].broadcast_to((P, 4, DH)), ALU.mult)

                    if ablate == "q":
                        # drain qt_g so the q pipeline has an observable effect
                        nc.sync.dma_start(out=out.ap()[512 * g:512 * g + P, 0:256],
                                          in_=qt_g[:, 0, :].bitcast(f32))
                        continue
                    do_ctx = ablate != "nosc"
                    for t in range(H // 2):
                        emit_scores_pair(t)
                        emit_exp(2 * t)
                        emit_exp(2 * t + 1)
                        if t >= 1 and do_ctx:
                            emit_ctx(2 * t - 2)
                            emit_ctx(2 * t - 1)
                    if do_ctx:
                        emit_ctx(H - 2)
                        emit_ctx(H - 1)
                        nc.sync.dma_start(
                            out=out.ap()[512 * g:512 * (g + 1), :]
                            .rearrange("(c p) d -> p c d", p=P),
                            in_=ctx_g[:])
                    else:
                        # drain probT so scores/exp aren't dead
                        nc.sync.dma_start(
                            out=out.ap()[512 * g:512 * g + P, 0:256],
                            in_=prob_tiles.pop(H - 1)[:, 0, :].bitcast(f32))

    nc.compile()
    return nc


_PROGRAM_CACHE = {}


def _get_program(S, D, KL):
    key = (S, D, KL)
    if key not in _PROGRAM_CACHE:
        _PROGRAM_CACHE[key] = build_program(S, D, KL)
    return _PROGRAM_CACHE[key]


def make_in_maps(hidden_states, attention_mask, Wq, bq, Wk, bk, Wv, bv,
                 proj_k, proj_v):
    """Host-side layout prep + batch sharding (1 sample per core)."""
    import ml_dtypes
    bf = ml_dtypes.bfloat16
    h = np.asarray(hidden_states, dtype=np.float32)
    Bn, S, D = h.shape
    scale = np.float32(1.0 / np.sqrt(DH))
    wqT = np.ascontiguousarray((np.asarray(Wq, np.float32) * scale).T).astype(bf)
    wkT = np.ascontiguousarray(np.asarray(Wk, np.float32).T).astype(bf)
    wvT = np.ascontiguousarray(np.asarray(Wv, np.float32).T).astype(bf)
    pkvn = np.concatenate([np.asarray(proj_k, np.float32)[:S],
                           np.asarray(proj_v, np.float32)[:S]], axis=1).astype(bf)
    bqn = (np.asarray(bq, np.float32) * scale).astype(np.float32)
    bkn = np.asarray(bk, np.float32).astype(bf)
    bvn = np.asarray(bv, np.float32).astype(bf)
    mask = np.asarray(attention_mask, np.float32).reshape(Bn, S)
    KL = pkvn.shape[1] // 2
    offs = _blob_offsets(S, D, KL)
    shared16 = [pkvn.reshape(-1), wqT.reshape(-1), wkT.reshape(-1),
                wvT.reshape(-1), bkn.reshape(-1), bvn.reshape(-1)]
    in_maps = []
    for b in range(Bn):
        hb = h[b]
        blob16 = np.concatenate(
            [hb.reshape(-1).astype(bf), hb.T.reshape(-1).astype(bf)] + shared16)
        assert blob16.shape[0] == offs["nb16"]
        blob32 = np.concatenate([mask[b], bqn]).astype(np.float32)
        assert blob32.shape[0] == offs["nb32"]
        in_maps.append(dict(blob16=blob16, blob32=blob32))
    return in_maps


def kernel(hidden_states, attention_mask, Wq, bq, Wk, bk, Wv, bv,
           proj_k, proj_v):
    h = np.asarray(hidden_states, dtype=np.float32)
    Bn, S, D = h.shape
    KL = np.asarray(proj_k).shape[1]
    nc = _get_program(S, D, KL)
    in_maps = make_in_maps(hidden_states, attention_mask, Wq, bq, Wk, bk,
                           Wv, bv, proj_k, proj_v)
    res = bass_utils.run_bass_kernel_spmd(nc, in_maps, core_ids=list(range(Bn)))
    return np.stack([res.results[b]["out"] for b in range(Bn)], axis=0)


def time_kernel(hidden_states, attention_mask, Wq, bq, Wk, bk, Wv, bv,
                proj_k, proj_v, k1=8, k2=40):
    """Estimate per-execution device time via pipelined-dispatch slope."""
    h = np.asarray(hidden_states, dtype=np.float32)
    S, D = h.shape[1], h.shape[2]
    KL = np.asarray(proj_k).shape[1]
    nc = _get_program(S, D, KL)
    in_maps = make_in_maps(hidden_states, attention_mask, Wq, bq, Wk, bk,
                           Wv, bv, proj_k, proj_v)
    return _time_nc(nc, in_maps, k1, k2)


def _time_nc(nc, in_maps, k1=8, k2=40):
    import time as _time
    import jax
    from jax.sharding import Mesh, PartitionSpec, NamedSharding
    from jax.experimental.shard_map import shard_map
    from concourse import bass2jax
    from concourse.bass2jax import _bass_exec_p, install_neuronx_cc_hook

    Bn = len(in_maps)
    install_neuronx_cc_hook()
    partition_name = nc.partition_id_tensor.name if nc.partition_id_tensor else None
    in_names, out_names, out_avals = [], [], []
    for alloc in nc.m.functions[0].allocations:
        if not isinstance(alloc, mybir.MemoryLocationSet):
            continue
        name = alloc.memorylocations[0].name
        if alloc.kind == "ExternalInput":
            if name != partition_name:
                in_names.append(name)
        elif alloc.kind == "ExternalOutput":
            out_names.append(name)
            out_avals.append(jax.core.ShapedArray(
                tuple(alloc.tensor_shape), mybir.dt.np(alloc.dtype)))
    n_params = len(in_names)
    all_in = list(in_names) + list(out_names)
    if partition_name is not None:
        all_in.append(partition_name)

    def _body(*args):
        operands = list(args)
        if partition_name is not None:
            operands.append(bass2jax.partition_id_tensor())
        return tuple(_bass_exec_p.bind(
            *operands, out_avals=tuple(out_avals), in_names=tuple(all_in),
            out_names=tuple(out_names), lowering_input_output_aliases=(),
            sim_require_finite=True, sim_require_nnan=True, nc=nc))

    devices = jax.devices()[:Bn]
    mesh = Mesh(np.asarray(devices), ("core",))
    fn = jax.jit(shard_map(_body, mesh=mesh,
                           in_specs=(PartitionSpec("core"),) * (n_params + len(out_names)),
                           out_specs=(PartitionSpec("core"),) * len(out_names),
                           check_rep=False), keep_unused=True)
    sh = NamedSharding(mesh, PartitionSpec("core"))
    dev_in = [jax.device_put(
        np.concatenate([in_maps[c][nm] for c in range(Bn)], axis=0), sh)
        for nm in in_names]
    zer = [jax.device_put(np.zeros((Bn * a.shape[0], *a.shape[1:]), a.dtype), sh)
           for a in out_avals]
    outs = fn(*dev_in, *zer)
    jax.block_until_ready(outs)

    def run(k):
        t0 = _time.time()
        rs = [fn(*dev_in, *zer) for _ in range(k)]
        jax.block_until_ready(rs)
        return _time.time() - t0

    run(3)  # warm
    # Load transients inflate wall times; a slope over per-k minima across
    # rounds is robust to them, and large k keeps the marginal-execution
    # signal well above the per-dispatch noise floor.
    ks = [10, 120]
    best = {k: np.inf for k in ks}
    for _ in range(3):
        for k in ks:
            best[k] = min(best[k], run(k))
    per_exec_s = (best[ks[1]] - best[ks[0]]) / (ks[1] - ks[0])
    return per_exec_s * 1e9


# revision 7
# speedup vs baseline: 1.2745x; 1.2745x over previous
"""Trainium2 Bass kernel v2 for Linformer self-attention (ragged projection).

Per batch sample b (data-parallel over 8 cores), with K=256 << S=4096 the
k/v projections are reordered to project h down to K rows FIRST:

    L        = sum(mask > -1);  mm[s] = (mask[s] > -1) / sqrt(L)
    pkvm     = [pk | pv] * mm[:, None]                  # [S, 2K]
    hpT      = h.T @ pkvm                               # [D, 2K]   phase 1
    cs       = ones.T @ (pkv * mm)                      # [1, 2K]   (bias csum)
    kT       = Wk.T.T @ hpT[:, :K] + bk x csk           # [D, K]    phase 2k
    v        = hpT[:, K:].T @ Wv.T + csv x bv           # [K, D]    phase 2v
    qT       = (Wq.T/sqrt(DH)).T @ h.T + bq/sqrt(DH)    # [D, S]    per group
    per head i:  scT = kT_i.T @ qT_i    [K, 512]        # heads row-packed x2
                 probT = exp(scT)  (unnormalized, bf16)
                 ctx[s, 64i+j] = probT.T @ [v_i | 1]; normalize by col 64

This cuts matmul flops ~1.9x vs computing full [S,D] hk/hv. All matmul
inputs are bf16 (f32 PSUM accumulation); DMA volume is halved vs f32.
ctx is computed in [s, d] layout (probT chunks as the stationary operand)
so the softmax denominator lands per-partition -> cheap tensor_scalar
normalization and contiguous output DMA.
"""

import numpy as np

import concourse.mybir as mybir
import concourse.tile as tile
from concourse import bacc
from concourse import bass_utils

P = 128
f32 = mybir.dt.float32
bf16 = mybir.dt.bfloat16
AF = mybir.ActivationFunctionType
ALU = mybir.AluOpType

# Problem dims (nn_LinformerSelfAttention): B=8, S=4096, D=1024, H=16, K=256
B = 8
S_FULL = 4096
D_FULL = 1024
KL_FULL = 256
DH = 64


def _blob_offsets(S, D, KL):
    """Element offsets of each packed input inside the two input blobs."""
    KV = 2 * KL
    offs = {}
    o = 0
    for name, n in [("hS", S * D), ("hT", D * S), ("pkv", S * KV),
                    ("wqT", D * D), ("wkT", D * D), ("wvT", D * D),
                    ("bkr", D), ("bvr", D)]:
        offs[name] = o
        o += n
    offs["nb16"] = o
    o = 0
    for name, n in [("mask", S), ("bqs", D)]:
        offs[name] = o
        o += n
    offs["nb32"] = o
    return offs


def build_program(S=S_FULL, D=D_FULL, KL=KL_FULL, ablate=None):
    """ablate: None=full, 'p12'=stop after phase2, 'q'=+q (no attention),
    'nosc'=+scores/exp (no ctx/norm/out)."""
    SC = S // P           # s-chunks of 128 (32)
    SG = S // 512         # s-groups of 512 (8)
    DC = D // P           # d-chunks of 128 (8)
    KC = KL // P          # linformer-k chunks of 128 (2)
    KV = 2 * KL           # fused [pk|pv] width (512)
    H = D // DH           # heads (16)
    HP = H // 2           # head pairs per d-chunk
    assert S % 512 == 0 and D % P == 0 and KL % P == 0 and KV <= 512

    nc = bacc.Bacc("TRN2", target_bir_lowering=False, debug=False)

    # All inputs are packed into two blobs (one bf16, one f32) to minimize
    # the per-execution dispatch cost, which scales with buffer count.
    offs = _blob_offsets(S, D, KL)
    blob16 = nc.dram_tensor("blob16", [offs["nb16"]], bf16, kind="ExternalInput")
    blob32 = nc.dram_tensor("blob32", [offs["nb32"]], f32, kind="ExternalInput")
    out = nc.dram_tensor("out", [S, D], f32, kind="ExternalOutput")

    def b16(name, n):
        o = offs[name]
        return blob16.ap()[o:o + n]

    def b32(name, n):
        o = offs[name]
        return blob32.ap()[o:o + n]

    with tile.TileContext(nc) as tc:
        with (
            tc.tile_pool(name="persist", bufs=1) as persist,
            tc.tile_pool(name="wpool", bufs=1) as wpool,
            tc.tile_pool(name="hpool", bufs=2) as hpool,
            tc.tile_pool(name="spool", bufs=4) as spool,
            tc.tile_pool(name="pkpool", bufs=8) as pkpool,
            tc.tile_pool(name="qpool", bufs=2) as qpool,
            tc.tile_pool(name="cpool", bufs=2) as cpool,
        ):
            # ---------- setup: mask stats ----------
            # pkvm needs only m01 = (mask > -1) (2 hops off the mask DMA);
            # the 1/sqrt(L) chain runs off the critical path and is folded
            # into the hp/cs PSUM drains at the end of phase 1.
            with tc.tile_pool(name="psetup", bufs=1, space="PSUM") as psetup:
                mt = spool.tile([P, SC], f32, tag="mt")
                nc.sync.dma_start(out=mt[:],
                                  in_=b32("mask", S).rearrange("(p c) -> p c", c=SC))
                m01 = persist.tile([P, SC], f32, tag="m01")
                nc.vector.tensor_scalar(m01[:], mt[:], -1.0, None, ALU.is_gt)
                lp = spool.tile([P, 1], f32, tag="lp")
                nc.vector.tensor_reduce(lp[:], m01[:], mybir.AxisListType.X, ALU.add)
                ones_col = persist.tile([P, 1], bf16, tag="onc")
                nc.vector.memset(ones_col[:], 1.0)
                ones_colf = spool.tile([P, 1], f32, tag="oncf")
                nc.vector.memset(ones_colf[:], 1.0)
                ones_row = spool.tile([1, P], f32, tag="onr")
                nc.vector.memset(ones_row[:], 1.0)
                lps = psetup.tile([1, 1], f32)
                nc.tensor.matmul(lps[:], lp[:], ones_colf[:], start=True, stop=True)
                lrec = spool.tile([1, 1], f32, tag="lrec")
                nc.vector.reciprocal(lrec[:], lps[:])
                inv = persist.tile([1, 1], f32, tag="inv")
                nc.scalar.activation(inv[:], lrec[:], AF.Sqrt)
                invps = psetup.tile([P, 1], f32)
                nc.tensor.matmul(invps[:], ones_row[:], inv[:], start=True, stop=True)
                invcol = persist.tile([P, 1], f32, tag="invcol")
                nc.vector.tensor_copy(invcol[:], invps[:])

            # ---------- phase 1: hpT[d, :] = sum_s h[s, d] * pkvm[s, :] -------
            # pkv chunks stream interleaved with hS chunks so the s-loop is
            # PE-bound from the start; pkv stays resident for the cs pass.
            pkv_sb = wpool.tile([P, SC, KV], bf16, tag="pkvsb", name="pkvsb")
            hp_sb = persist.tile([P, DC, KV], bf16, tag="hpsb")
            with tc.tile_pool(name="p1", bufs=1, space="PSUM") as p1:
                hp_ps = [p1.tile([P, KV], f32, tag=f"hp{d}", name=f"hpps{d}")
                         for d in range(DC)]
                csacc = persist.tile([P, KV], f32, tag="csacc")
                for G in range(SC // 4):
                    h_g = hpool.tile([P, 4, D], bf16, tag="hc")
                    # split the very first chunk off so the PE pipeline can
                    # start ~3us earlier than a full 4-chunk group allows
                    splits = [(0, 1), (1, 4)] if G == 0 else [(0, 4)]
                    for lo, hi in splits:
                        nc.sync.dma_start(
                            out=pkv_sb[:, 4 * G + lo:4 * G + hi, :],
                            in_=blob16.ap()[offs["pkv"] + (512 * G + 128 * lo) * KV:
                                            offs["pkv"] + (512 * G + 128 * hi) * KV]
                            .rearrange("(c p k) -> p c k", p=P, k=KV))
                        nc.sync.dma_start(
                            out=h_g[:, lo:hi, :],
                            in_=blob16.ap()[offs["hS"] + (512 * G + 128 * lo) * D:
                                            offs["hS"] + (512 * G + 128 * hi) * D]
                            .rearrange("(c p d) -> p c d", p=P, d=D))
                    # all four pkvm tiles first so the PE never starves on rhs
                    # (mask-only scale; 1/sqrt(L) is folded into the drains)
                    pkvms = []
                    for c in range(4):
                        s = 4 * G + c
                        pkvm = pkpool.tile([P, KV], bf16, tag="pkvm")
                        nc.vector.tensor_scalar(pkvm[:], pkv_sb[:, s, :],
                                                m01[:, s:s + 1], None, ALU.mult)
                        pkvms.append(pkvm)
                    for c in range(4):
                        s = 4 * G + c
                        for d in range(DC):
                            nc.tensor.matmul(hp_ps[d][:], h_g[:, c, P * d:P * (d + 1)],
                                             pkvms[c][:], start=(s == 0),
                                             stop=(s == SC - 1))
                    # per-partition partial csums for the bias rank-1 term,
                    # on the otherwise-idle Pool engine
                    for c in range(4):
                        s = 4 * G + c
                        if s == 0:
                            nc.gpsimd.tensor_copy(csacc[:], pkvms[c][:])
                        else:
                            nc.gpsimd.tensor_tensor(csacc[:], csacc[:],
                                                    pkvms[c][:], ALU.add)
                # drain hp PSUM banks in parallel on ACT and DVE, folding in
                # the 1/sqrt(L) scale (invcol is constant across partitions)
                for d in range(DC):
                    if d % 2 == 0:
                        nc.scalar.activation(hp_sb[:, d, :], hp_ps[d][:],
                                             AF.Copy, scale=invcol[:])
                    else:
                        nc.vector.tensor_scalar(hp_sb[:, d, :], hp_ps[d][:],
                                                invcol[:], None, ALU.mult)

            # biases + phase-2/q weights: DMAs queue behind the hS stream
            # (dispatch is ~650ns/DMA on the sync sequencer — keep the
            # startup-critical mask/pkv/hS DMAs in front)
            bq_sb = persist.tile([P, DC], f32, tag="bqsb")
            nc.sync.dma_start(out=bq_sb[:], in_=b32("bqs", D).rearrange("(m p) -> p m", p=P))
            bk_row = persist.tile([1, D], bf16, tag="bkrow")
            nc.sync.dma_start(out=bk_row[:], in_=b16("bkr", D)[None, :])
            bv_row = persist.tile([1, D], bf16, tag="bvrow")
            nc.sync.dma_start(out=bv_row[:], in_=b16("bvr", D)[None, :])
            wk_sb = wpool.tile([P, DC, D], bf16, tag="wk", name="wksb")
            wv_sb = wpool.tile([P, DC, D], bf16, tag="wv", name="wvsb")
            nc.sync.dma_start(out=wk_sb[:],
                              in_=b16("wkT", D * D)
                              .rearrange("(c p d) -> p c d", p=P, d=D))
            nc.sync.dma_start(out=wv_sb[:],
                              in_=b16("wvT", D * D)
                              .rearrange("(c p d) -> p c d", p=P, d=D))

            # ---------- cs = [csk | csv] = sum_s mm[s] * pkv[s, :] ------------
            # partition-reduce the DVE-accumulated csacc with one matmul
            cs_sb = persist.tile([1, KV], bf16, tag="cssb")
            csacc_bf = spool.tile([P, KV], bf16, tag="csaccbf")
            nc.vector.tensor_copy(csacc_bf[:], csacc[:])
            with tc.tile_pool(name="pcs", bufs=1, space="PSUM") as pcs:
                cs_ps = pcs.tile([1, KV], f32, name="csps")
                nc.tensor.matmul(cs_ps[:], ones_col[:], csacc_bf[:],
                                 start=True, stop=True)
                nc.vector.tensor_scalar(cs_sb[:], cs_ps[:], inv[:], None,
                                        ALU.mult)

            # ---------- phase 2k: kT[d, k] = sum_d' wkT[d', d] hpT[d', k] ----
            #            (+ bk[d] * csk[k])
            kt_sb = persist.tile([P, DC, KL], bf16, tag="ktsb")
            # ---------- phase 2v: v[k, d] = sum_d' hpT[d', K+k] wvT[d', d] ---
            #            (+ csv[k] * bv[d]) ; stored as vaug [k, dh+1] per head
            vaug = persist.tile([P, H * KC, DH + 1], bf16, tag="vaug")
            nc.vector.memset(vaug[:, :, DH:DH + 1], 1.0)
            D5 = (D + 511) // 512  # 512-wide column groups of D
            with tc.tile_pool(name="p2", bufs=1, space="PSUM") as p2:
                kt_ps = [p2.tile([P, 2 * KL], f32, tag=f"kt{j}", name=f"ktps{j}")
                         for j in range(DC // 2)]
                v_ps = [p2.tile([P, 512], f32, tag=f"v{j}", name=f"vps{j}")
                        for j in range(KC * D5)]
                HJ = 512 // DH  # heads per 512-wide v column group
                for kc in range(KC):
                    for j in range(D5):
                        o = v_ps[kc * D5 + j][:]
                        for dp in range(DC):
                            nc.tensor.matmul(
                                o, hp_sb[:, dp, KL + P * kc:KL + P * (kc + 1)],
                                wv_sb[:, dp, 512 * j:512 * (j + 1)],
                                start=(dp == 0), stop=False)
                        nc.tensor.matmul(
                            o, cs_sb[:, KL + P * kc:KL + P * (kc + 1)],
                            bv_row[:, 512 * j:512 * (j + 1)],
                            start=False, stop=True)
                        for i in range(HJ * j, min(HJ * (j + 1), H)):
                            src = v_ps[kc * D5 + j][:, DH * (i - HJ * j):
                                                    DH * (i - HJ * j) + DH]
                            if i % 2 == 0:
                                nc.scalar.activation(vaug[:, i * KC + kc, 0:DH],
                                                     src, AF.Copy)
                            else:
                                nc.vector.tensor_copy(vaug[:, i * KC + kc, 0:DH],
                                                      src)

                for d in range(DC):
                    o = kt_ps[d // 2][:, (d % 2) * KL:(d % 2) * KL + KL]
                    for dp in range(DC):
                        nc.tensor.matmul(o, wk_sb[:, dp, P * d:P * (d + 1)],
                                         hp_sb[:, dp, 0:KL],
                                         start=(dp == 0), stop=False)
                    nc.tensor.matmul(o, bk_row[:, P * d:P * (d + 1)],
                                     cs_sb[:, 0:KL], start=False, stop=True)
                # kt drains emitted before the v chains: the copies run under
                # the v matmuls (split ACT/DVE), freeing PSUM for the q phase.
                for d in range(DC):
                    src = kt_ps[d // 2][:, (d % 2) * KL:(d % 2) * KL + KL]
                    if d % 2 == 0:
                        nc.scalar.activation(kt_sb[:, d, :], src, AF.Copy)
                    else:
                        nc.vector.tensor_copy(kt_sb[:, d, :], src)
            # ---------- q + attention, per 512-group ----------
            if ablate == "p12":
                # drain kt so the phase-2 pipeline has an observable effect
                nc.sync.dma_start(out=out.ap()[0:P, 0:KL // 2],
                                  in_=kt_sb[:, 0, :].bitcast(f32))
            wq_sb = wpool.tile([P, DC, D], bf16, tag="wq", name="wqsb")
            if ablate != "p12":
                nc.sync.dma_start(out=wq_sb[:],
                                  in_=b16("wqT", D * D)
                                  .rearrange("(c p d) -> p c d", p=P, d=D))
            with (
                tc.tile_pool(name="pq", bufs=2, space="PSUM") as pq,
                tc.tile_pool(name="psc", bufs=4, space="PSUM") as psc,
                tc.tile_pool(name="pctx", bufs=2, space="PSUM") as pctx,
            ):
                for g in range(SG if ablate != "p12" else 0):
                    ht_g = hpool.tile([P, DC, 512], bf16, tag="ht")
                    nc.sync.dma_start(
                        out=ht_g[:],
                        in_=b16("hT", D * S)
                        .rearrange("(c p s) -> p c s", p=P, s=S)
                        [:, :, 512 * g:512 * (g + 1)])
                    qt_g = qpool.tile([P, DC, 512], bf16, tag="qt")
                    for mq in range(DC):
                        q_ps = pq.tile([P, 512], f32, tag="qps")
                        for d in range(DC):
                            nc.tensor.matmul(q_ps[:],
                                             wq_sb[:, d, P * mq:P * (mq + 1)],
                                             ht_g[:, d, :],
                                             start=(d == 0), stop=(d == DC - 1))
                        # q + bq/sqrt(DH); bias varies along partitions.
                        # On DVE: the ACT engine is near-saturated with exp.
                        nc.vector.tensor_scalar(qt_g[:, mq, :], q_ps[:],
                                                bq_sb[:, mq:mq + 1], None, ALU.add)
                    ctx_g = cpool.tile([P, 4, D], f32, tag="ctxg")
                    # software-pipelined head pairs: scores(j) ahead of ctx(j-1)
                    sc_tiles = {}
                    prob_tiles = {}

                    def emit_scores_pair(t):
                        # heads (2t, 2t+1) live at partition offsets 0/64 of
                        # d-chunk t; interleave their MMs so the row-tiled
                        # halves of the PE array run them concurrently.
                        # One PSUM tile per (head, kc): buffers recycle at
                        # exp granularity, not head granularity.
                        mq = t
                        for kc in range(KC):
                            for hh in range(2):
                                po = DH * hh
                                sc = psc.tile([P, 512], f32, tag="sc",
                                              name=f"scps{hh}{kc}")
                                nc.tensor.matmul(
                                    sc[:],
                                    kt_sb[po:po + DH, mq, P * kc:P * (kc + 1)],
                                    qt_g[po:po + DH, mq, :], start=True, stop=True)
                                sc_tiles[(2 * t + hh, kc)] = sc

                    def emit_exp(j):
                        probT = spool.tile([P, KC, 512], bf16, tag="probT")
                        for kc in range(KC):
                            sc = sc_tiles.pop((j, kc))
                            nc.scalar.activation(probT[:, kc, :], sc[:],
                                                 AF.Exp)
                        prob_tiles[j] = probT

                    def emit_ctx(j):
                        i = j  # head index
                        probT = prob_tiles.pop(j)
                        ctx_ps = pctx.tile([P, 4, DH + 1], f32, tag="cx")
                        for c in range(4):
                            for kc in range(KC):
                                nc.tensor.matmul(
                                    ctx_ps[:, c, :],
                                    probT[:, kc, P * c:P * (c + 1)],
                                    vaug[:, i * KC + kc, :],
                                    start=(kc == 0), stop=(kc == KC - 1))
                        rec4 = spool.tile([P, 4, 1], f32, tag="rec4")
                        nc.vector.reciprocal(rec4[:], ctx_ps[:, :, DH:DH + 1])
                        nc.vector.tensor_tensor(
                            ctx_g[:, :, DH * i:DH * (i + 1)],
                            ctx_ps[:, :, 0:DH],
                            rec4[:].broadcast_to((P, 4, DH)), ALU.mult)

                    if ablate == "q":
                        # drain qt_g so the q pipeline has an observable effect
                        nc.sync.dma_start(out=out.ap()[512 * g:512 * g + P, 0:256],
                                          in_=qt_g[:, 0, :].bitcast(f32))
                        continue
                    do_ctx = ablate != "nosc"
                    for t in range(H // 2):
                        emit_scores_pair(t)
                        emit_exp(2 * t)
                        emit_exp(2 * t + 1)
                        if t >= 1 and do_ctx:
                            emit_ctx(2 * t - 2)
                            emit_ctx(2 * t - 1)
                        # ship finished output quarters while later heads
                        # still compute (head 2t-1 is the last normalized)
                        if do_ctx and t % (H // 8) == 0 and 1 <= t // (H // 8) <= 3:
                            qi = t // (H // 8) - 1
                            lo, hi = qi * (D // 4), (qi + 1) * (D // 4)
                            nc.sync.dma_start(
                                out=out.ap()[512 * g:512 * (g + 1), lo:hi]
                                .rearrange("(c p) d -> p c d", p=P),
                                in_=ctx_g[:, :, lo:hi])
                    if do_ctx:
                        emit_ctx(H - 2)
                        emit_ctx(H - 1)
                        nc.sync.dma_start(
                            out=out.ap()[512 * g:512 * (g + 1), 3 * D // 4:]
                            .rearrange("(c p) d -> p c d", p=P),
                            in_=ctx_g[:, :, 3 * D // 4:])
                    else:
                        # drain probT so scores/exp aren't dead
                        nc.sync.dma_start(
                            out=out.ap()[512 * g:512 * g + P, 0:256],
                            in_=prob_tiles.pop(H - 1)[:, 0, :].bitcast(f32))

    nc.compile()
    return nc


_PROGRAM_CACHE = {}


def _get_program(S, D, KL):
    key = (S, D, KL)
    if key not in _PROGRAM_CACHE:
        _PROGRAM_CACHE[key] = build_program(S, D, KL)
    return _PROGRAM_CACHE[key]


def make_in_maps(hidden_states, attention_mask, Wq, bq, Wk, bk, Wv, bv,
                 proj_k, proj_v):
    """Host-side layout prep + batch sharding (1 sample per core)."""
    import ml_dtypes
    bf = ml_dtypes.bfloat16
    h = np.asarray(hidden_states, dtype=np.float32)
    Bn, S, D = h.shape
    scale = np.float32(1.0 / np.sqrt(DH))
    wqT = np.ascontiguousarray((np.asarray(Wq, np.float32) * scale).T).astype(bf)
    wkT = np.ascontiguousarray(np.asarray(Wk, np.float32).T).astype(bf)
    wvT = np.ascontiguousarray(np.asarray(Wv, np.float32).T).astype(bf)
    pkvn = np.concatenate([np.asarray(proj_k, np.float32)[:S],
                           np.asarray(proj_v, np.float32)[:S]], axis=1).astype(bf)
    bqn = (np.asarray(bq, np.float32) * scale).astype(np.float32)
    bkn = np.asarray(bk, np.float32).astype(bf)
    bvn = np.asarray(bv, np.float32).astype(bf)
    mask = np.asarray(attention_mask, np.float32).reshape(Bn, S)
    KL = pkvn.shape[1] // 2
    offs = _blob_offsets(S, D, KL)
    shared16 = [pkvn.reshape(-1), wqT.reshape(-1), wkT.reshape(-1),
                wvT.reshape(-1), bkn.reshape(-1), bvn.reshape(-1)]
    in_maps = []
    for b in range(Bn):
        hb = h[b]
        blob16 = np.concatenate(
            [hb.reshape(-1).astype(bf), hb.T.reshape(-1).astype(bf)] + shared16)
        assert blob16.shape[0] == offs["nb16"]
        # mask pre-transposed to the [P, SC] on-chip layout (contiguous DMA)
        mask_t = np.ascontiguousarray(mask[b].reshape(S // 128, 128).T).reshape(-1)
        blob32 = np.concatenate([mask_t, bqn]).astype(np.float32)
        assert blob32.shape[0] == offs["nb32"]
        in_maps.append(dict(blob16=blob16, blob32=blob32))
    return in_maps


def kernel(hidden_states, attention_mask, Wq, bq, Wk, bk, Wv, bv,
           proj_k, proj_v):
    h = np.asarray(hidden_states, dtype=np.float32)
    Bn, S, D = h.shape
    KL = np.asarray(proj_k).shape[1]
    nc = _get_program(S, D, KL)
    in_maps = make_in_maps(hidden_states, attention_mask, Wq, bq, Wk, bk,
                           Wv, bv, proj_k, proj_v)
    res = bass_utils.run_bass_kernel_spmd(nc, in_maps, core_ids=list(range(Bn)))
    return np.stack([res.results[b]["out"] for b in range(Bn)], axis=0)


def time_kernel(hidden_states, attention_mask, Wq, bq, Wk, bk, Wv, bv,
                proj_k, proj_v, k1=8, k2=40):
    """Estimate per-execution device time via pipelined-dispatch slope."""
    h = np.asarray(hidden_states, dtype=np.float32)
    S, D = h.shape[1], h.shape[2]
    KL = np.asarray(proj_k).shape[1]
    nc = _get_program(S, D, KL)
    in_maps = make_in_maps(hidden_states, attention_mask, Wq, bq, Wk, bk,
                           Wv, bv, proj_k, proj_v)
    return _time_nc(nc, in_maps, k1, k2)


def _time_nc(nc, in_maps, k1=8, k2=40):
    import time as _time
    import jax
    from jax.sharding import Mesh, PartitionSpec, NamedSharding
    from jax.experimental.shard_map import shard_map
    from concourse import bass2jax
    from concourse.bass2jax import _bass_exec_p, install_neuronx_cc_hook

    Bn = len(in_maps)
    install_neuronx_cc_hook()
    partition_name = nc.partition_id_tensor.name if nc.partition_id_tensor else None
    in_names, out_names, out_avals = [], [], []
    for alloc in nc.m.functions[0].allocations:
        if not isinstance(alloc, mybir.MemoryLocationSet):
            continue
        name = alloc.memorylocations[0].name
        if alloc.kind == "ExternalInput":
            if name != partition_name:
                in_names.append(name)
        elif alloc.kind == "ExternalOutput":
            out_names.append(name)
            out_avals.append(jax.core.ShapedArray(
                tuple(alloc.tensor_shape), mybir.dt.np(alloc.dtype)))
    n_params = len(in_names)
    all_in = list(in_names) + list(out_names)
    if partition_name is not None:
        all_in.append(partition_name)

    def _body(*args):
        operands = list(args)
        if partition_name is not None:
            operands.append(bass2jax.partition_id_tensor())
        return tuple(_bass_exec_p.bind(
            *operands, out_avals=tuple(out_avals), in_names=tuple(all_in),
            out_names=tuple(out_names), lowering_input_output_aliases=(),
            sim_require_finite=True, sim_require_nnan=True, nc=nc))

    devices = jax.devices()[:Bn]
    mesh = Mesh(np.asarray(devices), ("core",))
    fn = jax.jit(shard_map(_body, mesh=mesh,
                           in_specs=(PartitionSpec("core"),) * (n_params + len(out_names)),
                           out_specs=(PartitionSpec("core"),) * len(out_names),
                           check_rep=False), keep_unused=True)
    sh = NamedSharding(mesh, PartitionSpec("core"))
    dev_in = [jax.device_put(
        np.concatenate([in_maps[c][nm] for c in range(Bn)], axis=0), sh)
        for nm in in_names]
    zer = [jax.device_put(np.zeros((Bn * a.shape[0], *a.shape[1:]), a.dtype), sh)
           for a in out_avals]
    outs = fn(*dev_in, *zer)
    jax.block_until_ready(outs)

    def run(k):
        t0 = _time.time()
        rs = [fn(*dev_in, *zer) for _ in range(k)]
        jax.block_until_ready(rs)
        return _time.time() - t0

    run(3)  # warm
    # Load transients inflate wall times; a slope over per-k minima across
    # rounds is robust to them, and large k keeps the marginal-execution
    # signal well above the per-dispatch noise floor.
    ks = [10, 120]
    best = {k: np.inf for k in ks}
    for _ in range(3):
        for k in ks:
            best[k] = min(best[k], run(k))
    per_exec_s = (best[ks[1]] - best[ks[0]]) / (ks[1] - ks[0])
    return per_exec_s * 1e9
